# revision 23
# baseline (speedup 1.0000x reference)
"""Trainium2 Bass kernel for nn_Attention_86638080295542.

Multi-head attention (12 heads, d=64) with the reference's v=k quirk:
    q = x @ Wq.T + bq ; k = x @ Wk.T + bk ; v = k
    out = softmax(q k^T / sqrt(d)) @ v ;  y = out @ Wo.T + bo

Sharding: batch (B=8) data-parallel across the 8 NeuronCores — core c
computes batch element c end-to-end, no collectives.

v2 design notes (why it looks like this):
  HW evidence (variant timing) showed the v1 per-head attention stream ran
  at ~539ns/matmul = the COLD-clock serialized LDW+MM cost, while the
  projection stream ran warm (~226ns/MM).  The attention stream's 64-row
  stationaries, 65-col PV stationaries and interleaved PE-mode transposes
  (which don't register as PE-busy for the HAM clock gate) kept the PE
  throttled.  v2 therefore:
  - processes the two heads of each 128-row projection chunk TOGETHER per
    j-block: scores for head A (rows 0:64, tile_position (0,0)) and head B
    (rows 64:128, tile_position (64,0)) are issued back-to-back so the two
    64-contraction matmuls run CONCURRENTLY in different PE row groups.
  - pads the PV stationary to a full [128,128] tile (vaug cols: 0:64 = v,
    col 64 = ones for the softmax row-sum, cols 65:128 = zeros).
  - moves the kT -> k-natural (vaug) transposes OFF the PE onto the DMA
    XBAR engine (dma_start_transpose), one instruction per head.
  Dataflow per pair hp (heads A=2hp, B=2hp+1), per j-block jb:
    sps[j,0:1024]=kA^T qA ; sps[j,1024:2048]=kB^T qB   (4 MMs, A||B packed)
    ptA = exp(scale*spsA) ; ptB = exp(scale*spsB)      (2 ACT instrs — the
       pacer: ACT is ~100% busy at 2x1147ns per j-block)
    PV is deferred one full pair-window (pts persist in SBUF) so the single
    PSUM accumulator chain pvA -> normA -> pvB -> normB never stalls the
    scores/exp pipeline.  Row 64 of the accumulator is the softmax row-sum
    (ones-column trick); normalization = reciprocal (DVE) + partition
    broadcast (Pool/GpSimd) + multiply into outT (DVE).
  PSUM: scores tile [128,2048] f32 (4 banks, single-buffered — subtile WAR
  deps against the two exp reads give per-head double buffering) + one PV
  accumulator [128,1024] (2 banks) + proj/outproj ring 2x[128,512] (2).
"""

from contextlib import ExitStack

import numpy as np

import concourse.bass as bass
import concourse.tile as tile
from concourse import bacc, mybir
from concourse import bass_utils

S = 1024          # sequence length
E = 768           # embed dim
H = 12            # heads
DH = 64           # head dim
P = 128           # partitions
KT = E // P       # 6 k-tiles over embed dim
ST = S // P       # 8 tiles over sequence
NCH = S // 512    # 2 free-dim chunks of 512 over sequence
SCALE = DH ** -0.5
NCORES = 8

F32 = mybir.dt.float32
BF16 = mybir.dt.bfloat16

# rowsum broadcast: 'gpsimd' = nc.gpsimd.partition_broadcast,
# 'dma' = DRAM round-trip with a 0-step partition AP on the gpsimd
# software-DGE queue.  Measured on HW: gpsimd 266-271us vs dma 294us —
# the SWDGE round-trip delays the norm muls and stalls the PV chain.
BCAST = 'gpsimd'


def _emit(nc, tc, ctx, iters=1, variant='full'):
    xT_d = nc.dram_tensor("xT", [E, S], BF16, kind="ExternalInput")
    WqT_d = nc.dram_tensor("WqT", [E, E], BF16, kind="ExternalInput")
    WkT_d = nc.dram_tensor("WkT", [E, E], BF16, kind="ExternalInput")
    WoT_d = nc.dram_tensor("WoT", [E, E], BF16, kind="ExternalInput")
    bq_d = nc.dram_tensor("bq", [E], F32, kind="ExternalInput")
    bk_d = nc.dram_tensor("bk", [E], F32, kind="ExternalInput")
    bo_d = nc.dram_tensor("bo", [E], F32, kind="ExternalInput")
    y_d = nc.dram_tensor("y", [S, E], F32, kind="ExternalOutput")

    Exp = mybir.ActivationFunctionType.Exp

    const = ctx.enter_context(tc.tile_pool(name="const", bufs=1))
    xt_pool = ctx.enter_context(tc.tile_pool(name="xt", bufs=1))
    outt_pool = ctx.enter_context(tc.tile_pool(name="outt", bufs=1))
    wq_pool = ctx.enter_context(tc.tile_pool(name="wq", bufs=1))
    wk_pool = ctx.enter_context(tc.tile_pool(name="wk", bufs=1))
    wo_pool = ctx.enter_context(tc.tile_pool(name="wo", bufs=1))
    qt_pool = ctx.enter_context(tc.tile_pool(name="qt", bufs=3))
    kt_pool = ctx.enter_context(tc.tile_pool(name="kt", bufs=3))
    pta_pool = ctx.enter_context(tc.tile_pool(name="pta", bufs=11))
    ptb_pool = ctx.enter_context(tc.tile_pool(name="ptb", bufs=14))
    ysb_pool = ctx.enter_context(tc.tile_pool(name="ysb", bufs=4))
    rc_pool = ctx.enter_context(tc.tile_pool(name="rc", bufs=2))
    rb_pool = ctx.enter_context(tc.tile_pool(name="rb", bufs=2))
    ps_sc = ctx.enter_context(tc.tile_pool(name="ps_sc", bufs=1, space="PSUM"))
    ps_w = ctx.enter_context(tc.tile_pool(name="ps_w", bufs=2, space="PSUM"))
    ps_pv = ctx.enter_context(tc.tile_pool(name="ps_pv", bufs=1, space="PSUM"))
    if BCAST == 'dma':
        dram_pool = ctx.enter_context(
            tc.tile_pool(name="dram", bufs=4, space="DRAM"))

    # ---- loop-invariant constants (outside the timing loop) ----
    bq_sb = const.tile([P, KT], F32, tag="bq")
    nc.sync.dma_start(bq_sb[:], bq_d.ap().rearrange("(t p) -> p t", p=P))
    bk_sb = const.tile([P, KT], F32, tag="bk")
    nc.sync.dma_start(bk_sb[:], bk_d.ap().rearrange("(t p) -> p t", p=P))
    # bo broadcast to all 128 partitions via a 0-step partition AP
    bo_bc = const.tile([P, E], F32, tag="bo")
    bo_ap = bo_d.ap()
    bo_bcast_src = bass.AP(bo_ap.tensor, bo_ap.offset, [[0, P], [1, E]])
    nc.sync.dma_start(bo_bc[:], bo_bcast_src)
    # vaug[j, jb, h, 0:64] = k_h natural (written per pair by DMA transpose),
    # col 64 = ones (rowsum trick), cols 65:128 = zeros (pad so the PV
    # stationary is a full 128x128 tile).  Ones/zeros are loop-invariant.
    vaug = const.tile([P, ST, H, P], BF16, tag="vaug")
    nc.vector.memset(vaug[:], 0.0)
    nc.vector.memset(vaug[:, :, :, DH:DH + 1], 1.0)

    if variant in ('noexp', 'attnonly', 'attnburst'):
        pta_c = const.tile([P, S], BF16, tag="ptac")
        nc.vector.memset(pta_c[:], 0.01)
        ptb_c = const.tile([P, S], BF16, tag="ptbc")
        nc.vector.memset(ptb_c[:], 0.01)
        scr = const.tile([P, 256], F32, tag="scr")
    if variant in ('attnonly', 'attnburst'):
        qp_c = const.tile([P, S], BF16, tag="qpc")
        nc.vector.memset(qp_c[:], 0.02)
        kp_c = const.tile([P, S], BF16, tag="kpc")
        nc.vector.memset(kp_c[:], 0.02)

    if iters > 1:
        ctx.enter_context(tc.For_i(0, iters, 1))

    # ---- input loads: xT first (needed in full by proj 0), Wq/Wk in
    # hp-column slices so proj hp only waits for slice hp, WoT last ----
    xT_sb = xt_pool.tile([P, KT, S], BF16, tag="xt")
    WqT_sb = wq_pool.tile([P, KT, E], BF16, tag="wq")
    WkT_sb = wk_pool.tile([P, KT, E], BF16, tag="wk")
    WoT_sb = wo_pool.tile([P, KT, E], BF16, tag="wo")
    xT_r = xT_d.ap().rearrange("(t p) s -> p t s", p=P)
    WqT_r = WqT_d.ap().rearrange("(t p) e -> p t e", p=P)
    WkT_r = WkT_d.ap().rearrange("(t p) e -> p t e", p=P)
    WoT_r = WoT_d.ap().rearrange("(t p) e -> p t e", p=P)
    nc.sync.dma_start(xT_sb[:, 0, :], xT_r[:, 0, :])
    nc.sync.dma_start(WqT_sb[:, :, 0:P], WqT_r[:, :, 0:P])
    nc.sync.dma_start(WkT_sb[:, :, 0:P], WkT_r[:, :, 0:P])
    for t in range(1, KT):
        nc.sync.dma_start(xT_sb[:, t, :], xT_r[:, t, :])
    for hp in range(1, KT):
        c0, c1 = P * hp, P * hp + P
        nc.sync.dma_start(WqT_sb[:, :, c0:c1], WqT_r[:, :, c0:c1])
        nc.sync.dma_start(WkT_sb[:, :, c0:c1], WkT_r[:, :, c0:c1])
    for t in range(KT):
        nc.sync.dma_start(WoT_sb[:, t, :], WoT_r[:, t, :])

    outT_sb = outt_pool.tile([P, KT, S], BF16, tag="outt")

    qps = [None] * KT
    kps = [None] * KT

    def proj_pieces(hp, which, c):
        """One projection chunk as 3 pieces of 2 accumulating mms each; the
        last piece evicts to SBUF with the bias add (DVE)."""
        W_sb, b_sb = (WqT_sb, bq_sb) if which == 'q' else (WkT_sb, bk_sb)
        out_sb = qps[hp] if which == 'q' else kps[hp]
        st = {}

        def piece(tp, first, last):
            def go():
                if first:
                    st['ps'] = ps_w.tile([P, 512], F32, tag="ps_w",
                                         name=f"pj_{which}{hp}_{c}")
                ps = st['ps']
                for t in tp:
                    nc.tensor.matmul(
                        ps[:],
                        W_sb[:, t, 128 * hp:128 * hp + 128],
                        xT_sb[:, t, 512 * c:512 * c + 512],
                        start=(t == 0), stop=(t == KT - 1),
                    )
                if last:
                    nc.vector.tensor_scalar_add(
                        out_sb[:, 512 * c:512 * c + 512], ps[:],
                        b_sb[:, hp:hp + 1])
            return go

        return [piece((0, 1), True, False), piece((2, 3), False, False),
                piece((4, 5), False, True)]

    def trans_piece(hp):
        """k_nat for heads (2hp, 2hp+1) via XBAR DMA transpose: per head one
        instruction [64, S] -> vaug[:, :, h, 0:64] (= [j, jb, d])."""
        def go():
            kp = kps[hp]
            for hh in range(2):
                h = 2 * hp + hh
                nc.sync.dma_start_transpose(
                    vaug[:, :, h, 0:DH], kp[DH * hh:DH * hh + DH, :])
        return go

    def alloc_qk(hp):
        if variant in ('attnonly', 'attnburst'):
            qps[hp] = qp_c
            kps[hp] = kp_c
            return
        qps[hp] = qt_pool.tile([P, S], BF16, tag="qt", name=f"qp_{hp}")
        kps[hp] = kt_pool.tile([P, S], BF16, tag="kt", name=f"kp_{hp}")

    # TWO persistent per-head scores psum tiles for the whole iteration
    # (2 banks each).  Dependency tracking is per-TILE, so scores_A(jb+1)
    # only WARs against exp_A(jb) — head A's chain pipelines under head B's
    # exp and the ACT stream stays saturated without extra banks.  (A fresh
    # pool allocation per j-block, or one merged A|B tile, makes the WAR
    # cover BOTH exps: ACT then paces the PE into frequent short idle gaps,
    # and every gap re-throttles the PE clock.)
    spsA = ps_sc.tile([P, S], F32, tag="ps_scA", name="spsA")
    spsB = ps_sc.tile([P, S], F32, tag="ps_scB", name="spsB")

    def scores_jb(hp, jb):
        """Scores + exp for pair hp, j-block jb.  A and B are row-tile packed
        so the two 64-contraction matmuls run concurrently on the PE."""
        qp, kp = qps[hp], kps[hp]
        for c in range(NCH):
            cs = slice(512 * c, 512 * c + 512)
            nc.tensor.matmul(
                spsA[:, cs],
                kp[0:DH, 128 * jb:128 * jb + 128],
                qp[0:DH, cs],
                start=True, stop=True, tile_position=(0, 0),
            )
            nc.tensor.matmul(
                spsB[:, cs],
                kp[DH:P, 128 * jb:128 * jb + 128],
                qp[DH:P, cs],
                start=True, stop=True, tile_position=(64, 0),
            )
        if variant in ('noexp', 'attnonly', 'attnburst'):
            # skeleton timing: cheap DVE readers free the psum tiles; PV
            # uses constant pt tiles (no ACT in the dependency chain)
            nc.vector.tensor_copy(scr[:, 2 * (jb % 8):2 * (jb % 8) + 1],
                                  spsA[:, 0:1])
            nc.vector.tensor_copy(scr[:, 2 * (jb % 8) + 1:2 * (jb % 8) + 2],
                                  spsB[:, 0:1])
            return pta_c, ptb_c
        pta = pta_pool.tile([P, S], BF16, tag="pta")
        ptb = ptb_pool.tile([P, S], BF16, tag="ptb")
        nc.scalar.activation(pta[:], spsA[:], Exp, scale=SCALE)
        nc.scalar.activation(ptb[:], spsB[:], Exp, scale=SCALE)
        return pta, ptb

    def pv_pieces(hp, pts):
        """Deferred PV for pair hp: pvA(8 jb) -> normA -> pvB(8 jb) -> normB.
        Returns the ordered piece list; each piece is a closure."""
        pieces = []
        st = {}

        def pv_jb(h, jb, half, first, last):
            def go():
                if first:
                    st['pv'] = ps_pv.tile([P, S], F32, tag="ps_pv",
                                          name=f"pv_{h}")
                pv = st['pv']
                pt = pts[jb][half]
                for c in range(NCH):
                    nc.tensor.matmul(
                        pv[:, 512 * c:512 * c + 512],
                        vaug[:, jb, h, :],
                        pt[:, 512 * c:512 * c + 512],
                        start=(jb == 0), stop=(jb == ST - 1),
                    )
            return go

        def norm_start(h):
            def go():
                pv = st['pv']
                rc = rc_pool.tile([1, S], F32, tag="rc", name=f"rc_{h}")
                rb = rb_pool.tile([DH, S], F32, tag="rb", name=f"rb_{h}")
                if BCAST == 'dma':
                    rd = dram_pool.tile([1, S], F32, tag="rd", name=f"rd_{h}")
                for c in range(NCH):
                    cs = slice(512 * c, 512 * c + 512)
                    nc.vector.reciprocal(rc[:, cs], pv[DH:DH + 1, cs])
                    if BCAST == 'gpsimd':
                        nc.gpsimd.partition_broadcast(rb[:, cs], rc[:, cs])
                    else:
                        # broadcast across the 64 head-dim partitions via a
                        # DRAM round-trip (0-step partition AP) on the gpsimd
                        # software-DGE queue — no Pool-engine compute, and no
                        # head-of-line risk on the SP/ACT hwdge queues
                        nc.gpsimd.dma_start(rd[:, cs], rc[:, cs])
                        rd_ap = rd[:, cs]
                        nc.gpsimd.dma_start(
                            rb[:, cs],
                            bass.AP(rd_ap.tensor, rd_ap.offset,
                                    [[0, DH], [1, 512]]))
                st['rb'] = rb
            return go

        def norm_mul(h, hp_, po):
            def go():
                pv, rb = st['pv'], st['rb']
                for c in range(NCH):
                    cs = slice(512 * c, 512 * c + 512)
                    nc.vector.tensor_mul(
                        outT_sb[po:po + DH, hp_, cs], pv[0:DH, cs], rb[:, cs])
            return go

        hA, hB = 2 * hp, 2 * hp + 1
        for jb in range(ST):
            pieces.append(pv_jb(hA, jb, 0, jb == 0, jb == ST - 1))
        pieces.append(norm_start(hA))
        pieces.append(norm_mul(hA, hp, 0))
        for jb in range(ST):
            pieces.append(pv_jb(hB, jb, 1, jb == 0, jb == ST - 1))
        pieces.append(norm_start(hB))
        pieces.append(norm_mul(hB, hp, DH))
        return pieces

    def noattn_pieces(hp):
        def memset_out():
            nc.vector.memset(outT_sb[:, hp, :], 0.01)
        return [memset_out]

    # ---- startup: pair 0 projections + transposes (nothing to hide under)
    alloc_qk(0)
    if variant not in ('attnonly', 'attnburst'):
        for pc in (proj_pieces(0, 'q', 0) + proj_pieces(0, 'q', 1)
                   + proj_pieces(0, 'k', 0) + proj_pieces(0, 'k', 1)):
            pc()
        trans_piece(0)()

    # ---- pair windows: window hp runs scores/exp(hp); fill = pv(hp-1) +
    # norm tails + proj/trans(hp+1) ----
    pend_pv = []             # pv piece list of pair hp-1
    for hp in range(KT):
        pv_list = pend_pv
        if hp + 1 < KT and variant in ('attnonly', 'attnburst'):
            alloc_qk(hp + 1)
            prj = []
        elif hp + 1 < KT:
            alloc_qk(hp + 1)
            q0 = proj_pieces(hp + 1, 'q', 0)
            q1 = proj_pieces(hp + 1, 'q', 1)
            k0 = proj_pieces(hp + 1, 'k', 0)
            k1 = proj_pieces(hp + 1, 'k', 1)
            prj = q0 + q1 + k0 + k1 + [trans_piece(hp + 1)]
        else:
            prj = []
        # proportional merge keeping both streams in order (pv chain pieces
        # spread across the window so the accumulator chain never bunches)
        nA, nB = len(pv_list), len(prj)
        fill = [None] * (nA + nB)
        ia, ib = 0, 0
        for k in range(nA + nB):
            if ia < nA and (ib >= nB or ia * nB <= ib * nA):
                fill[k] = pv_list[ia]; ia += 1
            else:
                fill[k] = prj[ib]; ib += 1

        pts = {}
        fi = 0
        per_slot = (len(fill) + ST - 1) // ST
        if variant == 'attnburst':
            per_slot = 0
        for jb in range(ST):
            if variant == 'noattn':
                pts[jb] = (None, None)
            else:
                pts[jb] = scores_jb(hp, jb)
            for _ in range(per_slot):
                if fi < len(fill):
                    fill[fi]()
                    fi += 1
        for f in fill[fi:]:
            f()

        if variant == 'noattn':
            for pc in noattn_pieces(hp):
                pc()
            pend_pv = []
        else:
            pend_pv = pv_pieces(hp, pts)

    # ---- tail window: pv of the last pair, then output projection ----
    for f in pend_pv:
        f()

    # ---- output projection: y = outT^T @ WoT + bo ----
    if variant in ('attnonly', 'attnburst'):
        return
    y_r = y_d.ap().rearrange("(st p) e -> st p e", p=P)
    for st_i in range(ST):
        ysb = ysb_pool.tile([P, E], F32, tag="ysb")
        for ni, n0 in enumerate((0, 384)):
            # alternate psum pools so outproj chunks pipeline (the scores
            # slots are free once the last exps have read them)
            sel = (2 * st_i + ni) % 4
            if sel == 0:
                yps = ps_sc.tile([P, 512], F32, tag="ps_scA",
                                 name=f"yp_{st_i}_{n0}")
            elif sel == 2:
                yps = ps_sc.tile([P, 512], F32, tag="ps_scB",
                                 name=f"yp_{st_i}_{n0}")
            else:
                yps = ps_w.tile([P, 512], F32, tag="ps_w",
                                name=f"yp_{st_i}_{n0}")
            for t in range(KT):
                nc.tensor.matmul(
                    yps[:, 0:384],
                    outT_sb[:, t, 128 * st_i:128 * st_i + 128],
                    WoT_sb[:, t, n0:n0 + 384],
                    start=(t == 0), stop=(t == KT - 1),
                )
            nc.vector.tensor_add(ysb[:, n0:n0 + 384], yps[:, 0:384],
                                 bo_bc[:, n0:n0 + 384])
        # stores ride the ACT hwdge queue so next iteration's input loads
        # on the sync queue are not serialized behind them
        nc.scalar.dma_start(y_r[st_i], ysb[:])


_NC_CACHE = {}


def build(iters=1, variant="full"):
    key = (iters, variant)
    nc = _NC_CACHE.get(key)
    if nc is None:
        nc = bacc.Bacc("TRN2", target_bir_lowering=False, debug=False)
        with tile.TileContext(nc) as tc, ExitStack() as ctx:
            _emit(nc, tc, ctx, iters=iters, variant=variant)
        nc.compile()
        _NC_CACHE[key] = nc
    return nc


def make_in_maps(x, Wq, bq, Wk, bk, Wo, bo):
    import ml_dtypes
    BF = ml_dtypes.bfloat16
    WqT = np.ascontiguousarray(np.asarray(Wq, dtype=np.float32).T).astype(BF)
    WkT = np.ascontiguousarray(np.asarray(Wk, dtype=np.float32).T).astype(BF)
    WoT = np.ascontiguousarray(np.asarray(Wo, dtype=np.float32).T).astype(BF)
    bq = np.ascontiguousarray(np.asarray(bq, dtype=np.float32))
    bk = np.ascontiguousarray(np.asarray(bk, dtype=np.float32))
    bo = np.ascontiguousarray(np.asarray(bo, dtype=np.float32))
    x = np.asarray(x, dtype=np.float32)
    return [
        {
            "xT": np.ascontiguousarray(x[c].T).astype(BF),
            "WqT": WqT, "WkT": WkT, "WoT": WoT,
            "bq": bq, "bk": bk, "bo": bo,
        }
        for c in range(NCORES)
    ]


def kernel(x, Wq, bq, Wk, bk, Wo, bo):
    nc = build()
    in_maps = make_in_maps(x, Wq, bq, Wk, bk, Wo, bo)
    res = bass_utils.run_bass_kernel_spmd(nc, in_maps, core_ids=list(range(NCORES)))
    return np.stack([res.results[c]["y"] for c in range(NCORES)]).astype(np.float32)
